# revision 5
# baseline (speedup 1.0000x reference)
"""Talking-heads attention (B=8, N=1024, D=768, H=12, dh=64) on 8 TRN2 cores.

Strategy: pure data-parallel — core b computes batch element b; no collectives.

Per-core math (reference):
    q = x @ W_q * scale ; k,v = split(x @ W_kv)
    dots[h]  = q_h @ k_h^T
    dots2[g] = sum_h mix_pre[h,g] * dots[h]          (pre-softmax talking heads)
    attn     = softmax(dots2)
    attn2[g] = sum_h mix_post[h,g] * attn[h]         (post-softmax talking heads)
    out[g]   = attn2[g] @ v_g ;  y = concat(out) @ W_out + b_out

Kernel structure:
  * pre-mix folds into QK^T:  dots2[g] = q @ (mixpre-scaled k)^T with full
    768-wide contraction (K~_g[m,c] = mix_pre[c//64, g] * k[m,c]).
  * softmax: exp via ACT, rowsum via all-ones matmul (broadcast built in),
    normalize on DVE.
  * post-mix via a 96x96 block-diagonal matmul on a head-interleaved layout:
    normalized attn tiles [m, n] are written to DRAM as
    attn_dram[h, jb, m0lo, mt, i] (j = mt*128 + jb*16 + m0lo), read back
    interleaved as A_int[(h,jb) 96, m0lo 16, i 512], multiplied by the
    constant block-diagonal lhsT m2bd[(h,jb),(g,jb)] = mix_post[h,g], and
    round-tripped back to per-head attn2 tiles [m, n] through DRAM.
  * attn@v is then per-head with full K=128 contraction (M=64, two heads
    packed into one PSUM bank via base-partition 64).

Host-side numpy pre/post: shard x by batch, transpose x, fold the 1/8
attention scale into W_q, build m2bd from mix_post, un-transpose y.
mix_pre coefficients are baked into the instruction stream as immediates.
"""

import numpy as np
from contextlib import ExitStack

import concourse.bass as bass
import concourse.tile as tile
from concourse import bacc
from concourse import bass_isa
from concourse import mybir
from concourse.bass_utils import run_bass_kernel_spmd

P = 128
N = 1024          # sequence length
D = 768           # model dim
H = 12            # heads
DH = 64           # head dim
CT = D // P       # 6 feature tiles
NB = N // P       # 8 row blocks
SCALE = DH ** -0.5
F32 = mybir.dt.float32
BF16 = mybir.dt.bfloat16


def _build_graph(mix_pre: np.ndarray, mix_post: np.ndarray) -> bass.Bass:
    nc = bacc.Bacc()

    xT_d = nc.declare_dram_parameter("xT", [D, N], BF16, isOutput=False)
    wq_d = nc.declare_dram_parameter("wq", [D, D], BF16, isOutput=False)
    wkv_d = nc.declare_dram_parameter("wkv", [D, 2 * D], BF16, isOutput=False)
    wout_d = nc.declare_dram_parameter("wout", [D, D], BF16, isOutput=False)
    bout_d = nc.declare_dram_parameter("bout", [D, 1], F32, isOutput=False)
    m2bd_d = nc.declare_dram_parameter("m2bd", [96, 96], BF16, isOutput=False)
    yT_d = nc.declare_dram_parameter("yT", [D, N], F32, isOutput=True)

    with ExitStack() as ctx:
        tc = ctx.enter_context(tile.TileContext(nc))

        consts = ctx.enter_context(tc.tile_pool(name="consts", bufs=1))
        psum = ctx.enter_context(tc.tile_pool(name="psum", bufs=3, space="PSUM"))
        psum_r = ctx.enter_context(tc.tile_pool(name="psum_r", bufs=1, space="PSUM"))
        dram = ctx.enter_context(tc.tile_pool(name="dram", bufs=1, space="DRAM"))

        ones_mat = consts.tile([P, P], BF16)
        nc.vector.memset(ones_mat, 1.0)

        bias_sb = consts.tile([P, CT], F32)
        for i in range(CT):
            nc.sync.dma_start(out=bias_sb[:, i : i + 1],
                              in_=bout_d[i * P : (i + 1) * P, :])

        m2bd_sb = consts.tile([96, 96], BF16)
        nc.sync.dma_start(out=m2bd_sb, in_=m2bd_d[:, :])

        # ---- persistent bf16 activations/weights --------------------------
        qT_bf = consts.tile([P, CT, N], BF16)     # q^T, features on partitions
        kT_bf = consts.tile([P, CT, N], BF16)     # k^T
        v_bf = consts.tile([P, NB, D], BF16)      # v, rows(m) on partitions
        wout_bf = consts.tile([P, CT, D], BF16)
        accT_bf = consts.tile([P, CT, N], BF16)   # (attn2 @ v)^T assembled

        # DRAM scratch for the head-interleave roundtrips (one per n-half)
        # layout [h, jb, m0lo, mt, i] with j = mt*128 + jb*16 + m0lo
        attn_dr = [dram.tile([H, 8, 16, NB, 512], BF16, name=f"attn_dr{s}")
                   for s in range(2)]
        attn2_dr = [dram.tile([H, 8, 16, NB, 512], BF16, name=f"attn2_dr{s}")
                    for s in range(2)]

        # ---- load bf16 inputs directly (host pre-converts to bf16) --------
        with tc.tile_pool(name="projpool", bufs=1) as projpool:
            xT_bf = projpool.tile([P, CT, N], BF16)
            wq_bf = projpool.tile([P, CT, D], BF16)
            wkv_bf = projpool.tile([P, CT, 2 * D], BF16)
            for i in range(CT):
                nc.sync.dma_start(out=xT_bf[:, i, :],
                                  in_=xT_d[i * P : (i + 1) * P, :])
                nc.sync.dma_start(out=wq_bf[:, i, :],
                                  in_=wq_d[i * P : (i + 1) * P, :])
                nc.sync.dma_start(out=wkv_bf[:, i, :],
                                  in_=wkv_d[i * P : (i + 1) * P, :])
                nc.sync.dma_start(out=wout_bf[:, i, :],
                                  in_=wout_d[i * P : (i + 1) * P, :])

            # ---- projections -------------------------------------------------
            for i in range(CT):
                for nch in range(2):
                    ps = psum.tile([P, 512], F32, tag="mm")
                    for c in range(CT):
                        nc.tensor.matmul(
                            out=ps,
                            lhsT=wq_bf[:, c, i * P : (i + 1) * P],
                            rhs=xT_bf[:, c, nch * 512 : (nch + 1) * 512],
                            start=(c == 0), stop=(c == CT - 1),
                        )
                    nc.scalar.copy(qT_bf[:, i, nch * 512 : (nch + 1) * 512], ps)
                    ps = psum.tile([P, 512], F32, tag="mm")
                    for c in range(CT):
                        nc.tensor.matmul(
                            out=ps,
                            lhsT=wkv_bf[:, c, i * P : (i + 1) * P],
                            rhs=xT_bf[:, c, nch * 512 : (nch + 1) * 512],
                            start=(c == 0), stop=(c == CT - 1),
                        )
                    nc.scalar.copy(kT_bf[:, i, nch * 512 : (nch + 1) * 512], ps)
            # v[m, j] = sum_c x[m,c] W_v[c, j]
            for mt in range(NB):
                for ech in range(2):
                    ps = psum.tile([P, 512], F32, tag="mm")
                    for c in range(CT):
                        nc.tensor.matmul(
                            out=ps[:, :384],
                            lhsT=xT_bf[:, c, mt * P : (mt + 1) * P],
                            rhs=wkv_bf[:, c, D + ech * 384 : D + (ech + 1) * 384],
                            start=(c == 0), stop=(c == CT - 1),
                        )
                    nc.scalar.copy(v_bf[:, mt, ech * 384 : (ech + 1) * 384],
                                       ps[:, :384])

        # ---- main attention: two n-halves of 512 query columns each --------
        work = ctx.enter_context(tc.tile_pool(name="work", bufs=2))
        ptpool = ctx.enter_context(tc.tile_pool(name="ptpool", bufs=3))
        spool = ctx.enter_context(tc.tile_pool(name="spool", bufs=1))
        aint_pool = ctx.enter_context(tc.tile_pool(name="aint", bufs=2))
        a2sb_pool = ctx.enter_context(tc.tile_pool(name="a2sb", bufs=2))
        a2t_pool = ctx.enter_context(tc.tile_pool(name="a2t", bufs=2))
        opool = ctx.enter_context(tc.tile_pool(name="opool", bufs=2))

        for half in range(2):
            nlo = half * 512
            # ---- QK^T with pre-mix folded in; softmax; stage to DRAM -------
            for h in range(H):
                ktg = work.tile([P, CT, N], BF16, tag="ktg")
                for c in range(CT):
                    nc.vector.tensor_scalar_mul(
                        ktg[0:DH, c, :], kT_bf[0:DH, c, :],
                        float(mix_pre[2 * c, h]))
                    nc.vector.tensor_scalar_mul(
                        ktg[DH:P, c, :], kT_bf[DH:P, c, :],
                        float(mix_pre[2 * c + 1, h]))

                ptile = ptpool.tile([P, NB, 512], BF16, tag="pt")
                for mt in range(NB):
                    ps_d = psum.tile([P, 512], F32, tag="mm")
                    for c in range(CT):
                        nc.tensor.matmul(
                            out=ps_d,
                            lhsT=ktg[:, c, mt * P : (mt + 1) * P],
                            rhs=qT_bf[:, c, nlo : nlo + 512],
                            start=(c == 0), stop=(c == CT - 1),
                        )
                    nc.scalar.activation(
                        ptile[:, mt, :], ps_d,
                        mybir.ActivationFunctionType.Exp)
                # row sums over m via all-ones matmul (broadcast built in)
                ps_rs = psum_r.tile([P, 512], F32, tag="rs")
                for mt in range(NB):
                    nc.tensor.matmul(
                        out=ps_rs,
                        lhsT=ones_mat,
                        rhs=ptile[:, mt, :],
                        start=(mt == 0), stop=(mt == NB - 1),
                    )
                recipS = spool.tile([P, 512], F32, tag="recipS")
                nc.vector.reciprocal_approx_fast(recipS, ps_rs)
                for mt in range(NB):
                    nc.vector.tensor_mul(
                        ptile[:, mt, :], ptile[:, mt, :], recipS)
                # legA: stage normalized attn to DRAM [h, jb, m0lo, mt, i]
                # src partitions iterate m0 = jb*16 + m0lo (jb = top 3 bits)
                nc.sync.dma_start(out=attn_dr[half][h], in_=ptile)

            # ---- post-mix on the head-interleaved layout -------------------
            for mt in range(NB):
                a_int = aint_pool.tile([96, 16, 512], BF16, tag="a_int")
                nc.sync.dma_start(out=a_int,
                                  in_=attn_dr[half][:, :, :, mt, :])
                a2_sb = a2sb_pool.tile([96, 16, 512], BF16, tag="a2_sb")
                for m0lo in range(16):
                    ps_m = psum.tile([96, 512], F32, tag="pm", bufs=2)
                    nc.tensor.matmul(
                        out=ps_m,
                        lhsT=m2bd_sb,
                        rhs=a_int[:, m0lo, :],
                        start=True, stop=True,
                    )
                    nc.scalar.copy(a2_sb[:, m0lo, :], ps_m)
                nc.sync.dma_start(out=attn2_dr[half][:, :, :, mt, :],
                                  in_=a2_sb)

            # ---- attn2 @ v, per head, K=128 full contraction ---------------
            for gp in range(H // 2):          # head pairs share a PSUM bank
                ps_o = psum.tile([P, 512], F32, tag="av", bufs=2)
                a2t0 = a2t_pool.tile([P, NB, 512], BF16, tag="a2t0")
                nc.sync.dma_start(out=a2t0, in_=attn2_dr[half][2 * gp])
                a2t1 = a2t_pool.tile([P, NB, 512], BF16, tag="a2t1")
                nc.sync.dma_start(out=a2t1, in_=attn2_dr[half][2 * gp + 1])
                for mt in range(NB):
                    nc.tensor.matmul(
                        out=ps_o[0:64, :],
                        lhsT=v_bf[:, mt, 2 * gp * DH : (2 * gp + 1) * DH],
                        rhs=a2t0[:, mt, :],
                        start=(mt == 0), stop=(mt == NB - 1),
                    )
                for mt in range(NB):
                    nc.tensor.matmul(
                        out=ps_o[64:128, :],
                        lhsT=v_bf[:, mt, (2 * gp + 1) * DH : (2 * gp + 2) * DH],
                        rhs=a2t1[:, mt, :],
                        start=(mt == 0), stop=(mt == NB - 1),
                    )
                nc.scalar.copy(accT_bf[:, gp, nlo : nlo + 512], ps_o)

        # ---- output projection: y^T = W_out^T @ accT + b ------------------
        for i in range(CT):
            for nch in range(2):
                ps = psum.tile([P, 512], F32, tag="mm")
                for c in range(CT):
                    nc.tensor.matmul(
                        out=ps,
                        lhsT=wout_bf[:, c, i * P : (i + 1) * P],
                        rhs=accT_bf[:, c, nch * 512 : (nch + 1) * 512],
                        start=(c == 0), stop=(c == CT - 1),
                    )
                y_sb = opool.tile([P, 512], F32, tag="y_sb")
                nc.vector.tensor_scalar_add(y_sb, ps, bias_sb[:, i : i + 1])
                nc.sync.dma_start(
                    out=yT_d[i * P : (i + 1) * P, nch * 512 : (nch + 1) * 512],
                    in_=y_sb)

    nc.finalize()
    return nc


def _make_m2bd(mix_post: np.ndarray) -> np.ndarray:
    """lhsT for the block-diagonal post-mix: out = lhsT.T @ rhs with
    rhs rows (h*8+jb) and out rows (g*8+jb)."""
    m2bd = np.zeros((96, 96), dtype=np.float32)
    for jb in range(8):
        m2bd[jb::8, jb::8] = mix_post          # [h*8+jb, g*8+jb] = [h, g]
    return m2bd


def kernel(x, W_q, W_kv, mix_pre, mix_post, W_out, b_out):
    x = np.asarray(x, dtype=np.float32)
    W_q = np.asarray(W_q, dtype=np.float32)
    W_kv = np.asarray(W_kv, dtype=np.float32)
    mix_pre = np.asarray(mix_pre, dtype=np.float32)
    mix_post = np.asarray(mix_post, dtype=np.float32)
    W_out = np.asarray(W_out, dtype=np.float32)
    b_out = np.asarray(b_out, dtype=np.float32)

    B = x.shape[0]
    nc = _build_graph(mix_pre, mix_post)

    import ml_dtypes
    bf = ml_dtypes.bfloat16
    wq_s = np.ascontiguousarray((W_q * SCALE).astype(bf))
    wkv_c = np.ascontiguousarray(W_kv.astype(bf))
    wout_c = np.ascontiguousarray(W_out.astype(bf))
    bout_c = np.ascontiguousarray(b_out.reshape(D, 1))
    m2bd_c = np.ascontiguousarray(_make_m2bd(mix_post).astype(bf))

    in_maps = []
    for b in range(B):
        in_maps.append({
            "xT": np.ascontiguousarray(x[b].T.astype(bf)),
            "wq": wq_s,
            "wkv": wkv_c,
            "wout": wout_c,
            "bout": bout_c,
            "m2bd": m2bd_c,
        })

    res = run_bass_kernel_spmd(nc, in_maps, core_ids=list(range(B)))
    out = np.stack([np.ascontiguousarray(res.results[b]["yT"].T)
                    for b in range(B)], axis=0)
    return out.astype(np.float32)


if __name__ == "__main__":
    rng = np.random.default_rng(0)
    x = rng.standard_normal((8, N, D), dtype=np.float32)
    W_q = rng.standard_normal((D, D), dtype=np.float32) * 0.02
    W_kv = rng.standard_normal((D, 2 * D), dtype=np.float32) * 0.02
    mp = rng.standard_normal((H, H), dtype=np.float32)
    mq = rng.standard_normal((H, H), dtype=np.float32)
    W_out = rng.standard_normal((D, D), dtype=np.float32) * 0.02
    b_out = np.zeros((D,), dtype=np.float32)
    y = kernel(x=x, W_q=W_q, W_kv=W_kv, mix_pre=mp, mix_post=mq,
               W_out=W_out, b_out=b_out)
    print(y.shape, y.dtype)
